# revision 1
# baseline (speedup 1.0000x reference)
"""Trainium2 Bass kernel for nn_Model_22677427323544.

The circuit is AngleEmbedding(adds) followed by a batch-independent gate
sequence (all remaining gates depend only on params/weights/params2), then
<Z_0>. Algebraically:

    out[b] = psi0_b^H (U^H Z0 U) psi0_b          U = fixed 512x512 unitary
    psi0_b = D r_b,  D = diag((-i)^popcount(j)),  r_b real (Kronecker of
             [cos(t_i/2), sin(t_i/2)] per wire, wire 0 = MSB)
    =>  out[b] = r_b^T A r_b,   A = Re(D^H U^H Z0 U D)  real symmetric.

Host precomputes A (O(1) w.r.t. batch — pure parameter folding). The device
kernel, data-parallel over 8 cores (1024 samples each):
  1. sin/cos of adds/2 via ScalarE Sin (double-angle from t/4 for range safety)
  2. builds r as a 9-step Kronecker product on VectorE (wires processed
     8..0 so each step appends at the MSB => contiguous inner runs)
  3. transposes r via TensorE; PSUM->SBUF copies on ScalarE cast to fp32r
  4. Y = r @ A on TensorE in fp32r (1 col/cycle, ~17-bit mantissa)
  5. out = rowsum(Y * r) fused in one VectorE scalar_tensor_tensor per group
"""
import numpy as np
import ml_dtypes

import concourse.bass as bass
import concourse.tile as tile
from concourse import bacc, mybir
from concourse import bass_utils

N_WIRES = 9
DIM = 1 << N_WIRES            # 512
N_CORES = 8
B = 8192
B_LOC = B // N_CORES          # 1024
P = 128                       # partitions
G = B_LOC // P                # 8 batch groups per partition
KT = DIM // P                 # 4 contraction chunks
F32 = mybir.dt.float32
F32R = mybir.dt.float32r

# ---------------------------------------------------------------------------
# Host-side parameter folding: A = Re(D^H U^H Z0 U D)
# ---------------------------------------------------------------------------

_X = np.array([[0, 1], [1, 0]], dtype=np.complex128)
_CNOT = np.array(
    [[1, 0, 0, 0], [0, 1, 0, 0], [0, 0, 0, 1], [0, 0, 1, 0]], dtype=np.complex128
)


def _rx(t):
    c, s = np.cos(t / 2), np.sin(t / 2)
    return np.array([[c, -1j * s], [-1j * s, c]])


def _ry(t):
    c, s = np.cos(t / 2), np.sin(t / 2)
    return np.array([[c, -s], [s, c]], dtype=np.complex128)


def _rz(t):
    return np.array([[np.exp(-0.5j * t), 0], [0, np.exp(0.5j * t)]])


def _rot(phi, theta, omega):
    return _rz(omega) @ _ry(theta) @ _rz(phi)


def _crz(t):
    return np.diag([1, 1, np.exp(-0.5j * t), np.exp(0.5j * t)]).astype(np.complex128)


def _crx(t):
    m = np.eye(4, dtype=np.complex128)
    m[2:, 2:] = _rx(t)
    return m


def _apply_1q(state, U, w):
    s = np.moveaxis(state, 1 + w, -1)
    s = np.einsum('ij,...j->...i', U, s)
    return np.moveaxis(s, -1, 1 + w)


def _apply_2q(state, U, c, t):
    s = np.moveaxis(state, (1 + c, 1 + t), (-2, -1))
    shp = s.shape
    s = s.reshape(shp[:-2] + (4,))
    s = np.einsum('ij,...j->...i', U, s)
    return np.moveaxis(s.reshape(shp), (-2, -1), (1 + c, 1 + t))


def _entangle_block(state, p):
    j = 0
    for i in range(N_WIRES):
        ip = (i + 1) % N_WIRES
        state = _apply_1q(state, _ry(p[j]), i)
        state = _apply_1q(state, _ry(p[j + 1]), ip)
        state = _apply_2q(state, _CNOT, i, ip)
        state = _apply_2q(state, _crz(p[j + 2]), i, ip)
        state = _apply_1q(state, _X, ip)
        state = _apply_2q(state, _crx(p[j + 3]), i, ip)
        j += 4
    return state


def _sel_layer(state, w, r):
    for i in range(N_WIRES):
        state = _apply_1q(state, _rot(w[i, 0], w[i, 1], w[i, 2]), i)
    for i in range(N_WIRES):
        state = _apply_2q(state, _CNOT, i, (i + r) % N_WIRES)
    return state


def _round_fp32r(x):
    """Round fp32 to the 2xbf16-decomposable subset (fp32r)."""
    hi = x.astype(ml_dtypes.bfloat16).astype(np.float32)
    lo = (x - hi).astype(ml_dtypes.bfloat16).astype(np.float32)
    return hi + lo


def _compute_A(params, weights, params2):
    params = np.asarray(params, np.float64)
    weights = np.asarray(weights, np.float64)
    params2 = np.asarray(params2, np.float64)
    state = np.eye(DIM, dtype=np.complex128).reshape((DIM,) + (2,) * N_WIRES)
    for l in range(3):
        state = _entangle_block(state, params[l * 36:(l + 1) * 36])
    for l in range(3):
        state = _sel_layer(state, weights[l], (l % (N_WIRES - 1)) + 1)
    for l in range(5):
        state = _entangle_block(state, params2[l * 36:(l + 1) * 36])
    U = state.reshape(DIM, DIM).T
    z = np.where(np.arange(DIM) < DIM // 2, 1.0, -1.0)
    M = U.conj().T @ (z[:, None] * U)
    pc = np.array([bin(j).count('1') for j in range(DIM)])
    d = (-1j) ** pc
    A = (np.conj(d)[:, None] * M * d[None, :]).real
    return _round_fp32r(np.ascontiguousarray(A, dtype=np.float32))


# ---------------------------------------------------------------------------
# Device program (per core: 1024 samples; sample index = p*G + g)
# ---------------------------------------------------------------------------

_PROGRAM = None


def _build_program():
    nc = bacc.Bacc("TRN2", target_bir_lowering=False, debug=False,
                   num_devices=N_CORES)
    adds_ext = nc.dram_tensor("adds", [B_LOC, N_WIRES], F32,
                              kind="ExternalInput").ap()
    amat_ext = nc.dram_tensor("amat", [DIM, DIM], F32R,
                              kind="ExternalInput").ap()
    out_ext = nc.dram_tensor("out", [B_LOC], F32, kind="ExternalOutput").ap()

    with tile.TileContext(nc) as tc:
        with (
            tc.tile_pool(name="const", bufs=1) as cpool,
            tc.tile_pool(name="work", bufs=2) as wpool,
            tc.tile_pool(name="psum_t", bufs=2, space="PSUM") as pt,
            tc.tile_pool(name="psum_y", bufs=4, space="PSUM") as py,
        ):
            # adds shard first (small, unblocks the whole front end)
            adds_sb = cpool.tile([P, G, N_WIRES], F32)
            nc.sync.dma_start(adds_sb[:], adds_ext.rearrange("(p g) i -> p g i", g=G))

            # A matrix (fp32r, host-rounded): amat_sb[k_lo, k_hi, n]
            amat_sb = cpool.tile([P, KT, DIM], F32R)
            a_view = amat_ext.rearrange("(kh kl) n -> kl kh n", kl=P)
            for kh in range(KT):
                nc.sync.dma_start(amat_sb[:, kh, :], a_view[:, kh, :])

            # identity for PE transpose
            ident = cpool.tile([P, P], F32)
            nc.gpsimd.memset(ident[:], 0.0)
            nc.gpsimd.affine_select(
                out=ident[:], in_=ident[:],
                compare_op=mybir.AluOpType.not_equal, fill=1.0,
                base=0, pattern=[[-1, P]], channel_multiplier=1)
            halfpi = cpool.tile([P, 1], F32)
            nc.vector.memset(halfpi[:], float(np.pi / 2))

            # u = sin(t/4), v = cos(t/4); c = 1-2u^2, s = 2uv
            u = cpool.tile([P, G, N_WIRES], F32)
            v = cpool.tile([P, G, N_WIRES], F32)
            nc.scalar.activation(u[:], adds_sb[:], mybir.ActivationFunctionType.Sin,
                                 scale=0.25)
            nc.scalar.activation(v[:], adds_sb[:], mybir.ActivationFunctionType.Sin,
                                 scale=-0.25, bias=halfpi[:])
            # cs[p, g, 0, i] = cos(t_i/2), cs[p, g, 1, i] = sin(t_i/2)
            cs = cpool.tile([P, G, 2, N_WIRES], F32)
            nc.vector.scalar_tensor_tensor(
                out=cs[:, :, 1, :], in0=u[:], scalar=2.0, in1=v[:],
                op0=mybir.AluOpType.mult, op1=mybir.AluOpType.mult)
            usq = cpool.tile([P, G, N_WIRES], F32)
            nc.vector.tensor_mul(usq[:], u[:], u[:])
            nc.vector.tensor_scalar(
                out=cs[:, :, 0, :], in0=usq[:], scalar1=-2.0, scalar2=1.0,
                op0=mybir.AluOpType.mult, op1=mybir.AluOpType.add)

            # Kronecker build of r, appending each new wire at the MSB:
            # process wires 8,7,...,0 so wire 0 ends up as the MSB (stride 256)
            # and wire 8 as the LSB — the reference flattening order.
            # step: out[p, g, b*L + m] = in[p, g, m] * cs[p, g, b, w]
            sA = cpool.tile([P, G, 128], F32)
            sB = cpool.tile([P, G, 256], F32)
            rmag = cpool.tile([P, G, DIM], F32)
            nc.vector.tensor_copy(sA[:, :, :2], cs[:, :, :, N_WIRES - 1])
            cur = sA
            for step in range(1, N_WIRES - 1):
                w = N_WIRES - 1 - step
                L = 1 << step
                nxt = sB if cur is sA else sA
                out_ap = nxt[:, :, :2 * L].rearrange("p g (b m) -> p g b m", b=2)
                in0 = cur[:, :, None, :L].to_broadcast((P, G, 2, L))
                in1 = cs[:, :, :, w][:, :, :, None].to_broadcast((P, G, 2, L))
                nc.vector.tensor_mul(out_ap, in0, in1)
                cur = nxt
            # last step (wire 0) split per group so downstream work pipelines
            HALF = DIM // 2
            for g in range(G):
                out_ap = rmag[:, g, :].rearrange("p (b m) -> p b m", b=2)
                in0 = cur[:, g, None, :].to_broadcast((P, 2, HALF))
                in1 = cs[:, g, :, 0][:, :, None].to_broadcast((P, 2, HALF))
                nc.vector.tensor_mul(out_ap, in0, in1)

            # Transpose to contraction layout (fp32 PE transpose), PSUM->SBUF
            # copy on ScalarE with cast to fp32r:
            # rmagT[j_lo, k, g*128 + p] = rmag[p, g, k*128 + j_lo]
            rmagT = cpool.tile([P, KT, B_LOC], F32R)
            res = cpool.tile([P, G], F32)
            for g in range(G):
                tp = pt.tile([P, DIM], F32, tag="tp")
                for k in range(KT):
                    nc.tensor.transpose(tp[:, k * P:(k + 1) * P],
                                        rmag[:, g, k * P:(k + 1) * P], ident[:])
                nc.scalar.copy(
                    rmagT[:, :, g * P:(g + 1) * P],
                    tp[:].rearrange("p (k x) -> p k x", k=KT))

                # Y_g = r_g @ A  (fp32r matmul, fp32 PSUM accumulate)
                yp = py.tile([P, DIM], F32, tag="yp")
                for k in range(KT):
                    nc.tensor.matmul(yp[:], lhsT=rmagT[:, k, g * P:(g + 1) * P],
                                     rhs=amat_sb[:, k, :],
                                     start=(k == 0), stop=(k == KT - 1))
                # out[:, g] = rowsum(Y_g * r_g), fused
                wscr = wpool.tile([P, DIM], F32, tag="wscr")
                nc.vector.scalar_tensor_tensor(
                    out=wscr[:], in0=yp[:], scalar=0.0, in1=rmag[:, g, :],
                    op0=mybir.AluOpType.add, op1=mybir.AluOpType.mult,
                    accum_out=res[:, g:g + 1])

            nc.sync.dma_start(out_ext.rearrange("(p g) -> p g", g=G), res[:])

    nc.compile()
    return nc


def _get_program():
    global _PROGRAM
    if _PROGRAM is None:
        _PROGRAM = _build_program()
    return _PROGRAM


def kernel(adds, params, weights, params2):
    adds = np.ascontiguousarray(np.asarray(adds), dtype=np.float32)
    A = _compute_A(params, weights, params2)
    nc = _get_program()
    in_maps = [
        {"adds": adds[i * B_LOC:(i + 1) * B_LOC], "amat": A}
        for i in range(N_CORES)
    ]
    results = bass_utils.run_bass_kernel_spmd(nc, in_maps, list(range(N_CORES))).results
    return np.concatenate([results[i]["out"] for i in range(N_CORES)])



# revision 3
# speedup vs baseline: 1.6357x; 1.6357x over previous
"""Trainium2 Bass kernel for nn_Model_22677427323544.

The circuit is AngleEmbedding(adds) followed by a batch-independent gate
sequence, then <Z_0>. Algebraically out[b] = r_b^T A r_b with A a fixed real
symmetric 512x512 matrix and r_b the real Kronecker vector of per-wire
(cos(t/2), sin(t/2)).

Key reduction: each wire contributes a factor c^2, s^2, or c*s to every
A[j,k] r_j r_k term, so the quadratic form collapses to a LINEAR form over
per-wire 3-vectors g_i = (c_i^2, s_i^2, c_i*s_i):

    out[b] = < A3 , g_0[b] x g_1[b] x ... x g_8[b] >

with A3 the 3^9 tensor A3[m] = sum_{(j,k) -> m} A[j,k] folded on host
(m_i = 0 for (j_i,k_i)=(0,0), 1 for (1,1), 2 for mixed).  Split wires 0-3
(81) / 4-8 (243):  out[b] = G_hi[b]^T A3 G_lo[b]  -- an [81,243] matvec per
sample: 13x fewer MACs and 13x less HBM than the 512x512 quadratic form.

Device (per core, 1024 samples = 128 partitions x 8 groups):
  1. sin/cos of t/4 via ScalarE; c,s (half-angle) and g-vectors on VectorE
  2. base-3 Kronecker builds of G_hi [P,G,81], G_lo [P,G,243] on VectorE
  3. per group: PE-transpose G_hi -> ScalarE copy (cast fp32r) ->
     TensorE matmul Y = G_hi^T @ A3 -> fused VectorE dot with G_lo
"""
import numpy as np
import ml_dtypes

import concourse.bass as bass
import concourse.tile as tile
from concourse import bacc, mybir
from concourse import bass_utils

N_WIRES = 9
N_CORES = 8
B = 8192
B_LOC = B // N_CORES          # 1024
P = 128                       # partitions
G = B_LOC // P                # 8 batch groups per partition
NH = 81                       # 3^4, wires 0-3
NL = 243                      # 3^5, wires 4-8
NLP = 256                     # NL padded: fp32r matmul needs even N; N>=256 for 1 cyc/row
F32 = mybir.dt.float32
F32R = mybir.dt.float32r

# ---------------------------------------------------------------------------
# Host-side parameter folding: A = Re(D^H U^H Z0 U D), then 3-ary fold
# ---------------------------------------------------------------------------

DIM = 1 << N_WIRES

_X = np.array([[0, 1], [1, 0]], dtype=np.complex128)
_CNOT = np.array(
    [[1, 0, 0, 0], [0, 1, 0, 0], [0, 0, 0, 1], [0, 0, 1, 0]], dtype=np.complex128
)


def _rx(t):
    c, s = np.cos(t / 2), np.sin(t / 2)
    return np.array([[c, -1j * s], [-1j * s, c]])


def _ry(t):
    c, s = np.cos(t / 2), np.sin(t / 2)
    return np.array([[c, -s], [s, c]], dtype=np.complex128)


def _rz(t):
    return np.array([[np.exp(-0.5j * t), 0], [0, np.exp(0.5j * t)]])


def _rot(phi, theta, omega):
    return _rz(omega) @ _ry(theta) @ _rz(phi)


def _crz(t):
    return np.diag([1, 1, np.exp(-0.5j * t), np.exp(0.5j * t)]).astype(np.complex128)


def _crx(t):
    m = np.eye(4, dtype=np.complex128)
    m[2:, 2:] = _rx(t)
    return m


def _apply_1q(state, U, w):
    s = np.moveaxis(state, 1 + w, -1)
    s = np.einsum('ij,...j->...i', U, s)
    return np.moveaxis(s, -1, 1 + w)


def _apply_2q(state, U, c, t):
    s = np.moveaxis(state, (1 + c, 1 + t), (-2, -1))
    shp = s.shape
    s = s.reshape(shp[:-2] + (4,))
    s = np.einsum('ij,...j->...i', U, s)
    return np.moveaxis(s.reshape(shp), (-2, -1), (1 + c, 1 + t))


def _entangle_block(state, p):
    j = 0
    for i in range(N_WIRES):
        ip = (i + 1) % N_WIRES
        state = _apply_1q(state, _ry(p[j]), i)
        state = _apply_1q(state, _ry(p[j + 1]), ip)
        state = _apply_2q(state, _CNOT, i, ip)
        state = _apply_2q(state, _crz(p[j + 2]), i, ip)
        state = _apply_1q(state, _X, ip)
        state = _apply_2q(state, _crx(p[j + 3]), i, ip)
        j += 4
    return state


def _sel_layer(state, w, r):
    for i in range(N_WIRES):
        state = _apply_1q(state, _rot(w[i, 0], w[i, 1], w[i, 2]), i)
    for i in range(N_WIRES):
        state = _apply_2q(state, _CNOT, i, (i + r) % N_WIRES)
    return state


def _round_fp32r(x):
    """Round fp32 to the 2xbf16-decomposable subset (fp32r)."""
    hi = x.astype(ml_dtypes.bfloat16).astype(np.float32)
    lo = (x - hi).astype(ml_dtypes.bfloat16).astype(np.float32)
    return hi + lo


def _compute_A(params, weights, params2):
    """Return the folded 3-ary coefficient matrix A3 [81, 243] (fp32r)."""
    params = np.asarray(params, np.float64)
    weights = np.asarray(weights, np.float64)
    params2 = np.asarray(params2, np.float64)
    state = np.eye(DIM, dtype=np.complex128).reshape((DIM,) + (2,) * N_WIRES)
    for l in range(3):
        state = _entangle_block(state, params[l * 36:(l + 1) * 36])
    for l in range(3):
        state = _sel_layer(state, weights[l], (l % (N_WIRES - 1)) + 1)
    for l in range(5):
        state = _entangle_block(state, params2[l * 36:(l + 1) * 36])
    U = state.reshape(DIM, DIM).T
    z = np.where(np.arange(DIM) < DIM // 2, 1.0, -1.0)
    M = U.conj().T @ (z[:, None] * U)
    pc = np.array([bin(j).count('1') for j in range(DIM)])
    d = (-1j) ** pc
    A = ((np.conj(d)[:, None] * M * d[None, :]).real).astype(np.float64)

    # fold 512x512 -> 3^9: digit 0 = (0,0), 1 = (1,1), 2 = (0,1)/(1,0)
    j = np.arange(DIM)
    jb = (j[:, None, None] >> (8 - np.arange(N_WIRES))[None, None, :]) & 1
    kb = (j[None, :, None] >> (8 - np.arange(N_WIRES))[None, None, :]) & 1
    digit = np.where((jb == 0) & (kb == 0), 0, np.where((jb == 1) & (kb == 1), 1, 2))
    m = np.zeros((DIM, DIM), np.int64)
    for i in range(N_WIRES):
        m = m * 3 + digit[:, :, i]
    A3 = np.zeros(3 ** N_WIRES)
    np.add.at(A3, m.ravel(), A.ravel())
    A3 = A3.reshape(NH, NL).astype(np.float32)
    A3p = np.zeros((NH, NLP), np.float32)
    A3p[:, :NL] = A3
    return _round_fp32r(np.ascontiguousarray(A3p))


# ---------------------------------------------------------------------------
# Device program (per core: 1024 samples; sample index = p*G + g)
# ---------------------------------------------------------------------------

_PROGRAM = None


def _build_program():
    nc = bacc.Bacc("TRN2", target_bir_lowering=False, debug=False,
                   num_devices=N_CORES)
    adds_ext = nc.dram_tensor("adds", [B_LOC, N_WIRES], F32,
                              kind="ExternalInput").ap()
    amat_ext = nc.dram_tensor("amat", [NH, NLP], F32R,
                              kind="ExternalInput").ap()
    out_ext = nc.dram_tensor("out", [B_LOC], F32, kind="ExternalOutput").ap()

    with tile.TileContext(nc) as tc:
        with (
            tc.tile_pool(name="const", bufs=1) as cpool,
            tc.tile_pool(name="work", bufs=2) as wpool,
            tc.tile_pool(name="psum_t", bufs=2, space="PSUM") as pt,
            tc.tile_pool(name="psum_y", bufs=4, space="PSUM") as py,
        ):
            # adds shard first (small, unblocks the whole front end)
            adds_sb = cpool.tile([P, G, N_WIRES], F32)
            nc.sync.dma_start(adds_sb[:], adds_ext.rearrange("(p g) i -> p g i", g=G))

            # A3 matrix (fp32r, host-rounded)
            a3_sb = cpool.tile([NH, NLP], F32R)
            nc.sync.dma_start(a3_sb[:], amat_ext)

            # identity for PE transpose
            ident = cpool.tile([P, P], F32)
            nc.gpsimd.memset(ident[:], 0.0)
            nc.gpsimd.affine_select(
                out=ident[:], in_=ident[:],
                compare_op=mybir.AluOpType.not_equal, fill=1.0,
                base=0, pattern=[[-1, P]], channel_multiplier=1)
            halfpi = cpool.tile([P, 1], F32)
            nc.vector.memset(halfpi[:], float(np.pi / 2))

            # u = sin(t/4), v = cos(t/4); c = 1-2u^2, s = 2uv (half-angle)
            u = cpool.tile([P, G, N_WIRES], F32)
            v = cpool.tile([P, G, N_WIRES], F32)
            nc.scalar.activation(u[:], adds_sb[:], mybir.ActivationFunctionType.Sin,
                                 scale=0.25)
            nc.scalar.activation(v[:], adds_sb[:], mybir.ActivationFunctionType.Sin,
                                 scale=-0.25, bias=halfpi[:])
            cc = cpool.tile([P, G, N_WIRES], F32)
            ss = cpool.tile([P, G, N_WIRES], F32)
            usq = cpool.tile([P, G, N_WIRES], F32)
            nc.vector.scalar_tensor_tensor(
                out=ss[:], in0=u[:], scalar=2.0, in1=v[:],
                op0=mybir.AluOpType.mult, op1=mybir.AluOpType.mult)
            nc.vector.tensor_mul(usq[:], u[:], u[:])
            nc.vector.tensor_scalar(
                out=cc[:], in0=usq[:], scalar1=-2.0, scalar2=1.0,
                op0=mybir.AluOpType.mult, op1=mybir.AluOpType.add)

            # per-wire g-vectors: gv[p,g,w,:] = (c^2, s^2, c*s)
            gv = cpool.tile([P, G, N_WIRES, 3], F32)
            nc.vector.tensor_mul(gv[:, :, :, 0], cc[:], cc[:])
            nc.vector.tensor_mul(gv[:, :, :, 1], ss[:], ss[:])
            nc.vector.tensor_mul(gv[:, :, :, 2], cc[:], ss[:])

            # base-3 Kronecker build; each step prepends the new wire at the
            # MSB digit:  out[p,g, b*L + m] = in[p,g,m] * gv[p,g,w,b]
            def kron_step(out_t, in_t, w, L, gslice=None):
                if gslice is None:
                    out_ap = out_t[:, :, :3 * L].rearrange(
                        "p g (b m) -> p g b m", b=3)
                    in0 = in_t[:, :, None, :L].to_broadcast((P, G, 3, L))
                    in1 = gv[:, :, w, :][:, :, :, None].to_broadcast((P, G, 3, L))
                else:
                    g_ = gslice
                    out_ap = out_t[:, g_, :3 * L].rearrange("p (b m) -> p b m", b=3)
                    in0 = in_t[:, g_, None, :L].to_broadcast((P, 3, L))
                    in1 = gv[:, g_, w, :][:, :, None].to_broadcast((P, 3, L))
                nc.vector.tensor_mul(out_ap, in0, in1)

            # G_hi: wires 3,2,1,0 -> [P, G, 81]
            h1 = cpool.tile([P, G, 9], F32)
            h2 = cpool.tile([P, G, 27], F32)
            ghi = cpool.tile([P, G, NH], F32)
            kron_step(h1, gv[:, :, 3, :], 2, 3)
            kron_step(h2, h1, 1, 9)
            kron_step(ghi, h2, 0, 27)

            # per-group: transpose G_hi, cast to fp32r, matmul with A3
            ghiT = cpool.tile([NH, G, P], F32R)
            yps = []
            for g in range(G):
                tp = pt.tile([NH, P], F32, tag="tp")
                nc.tensor.transpose(tp[:], ghi[:, g, :], ident[:])
                nc.scalar.copy(ghiT[:, g, :], tp[:])
                yp = py.tile([P, NLP], F32, tag="yp")
                nc.tensor.matmul(yp[:], lhsT=ghiT[:, g, :], rhs=a3_sb[:],
                                 start=True, stop=True)
                yps.append(yp)

            # G_lo: wires 8,7,6,5 shared, wire 4 split per group
            l1 = cpool.tile([P, G, 9], F32)
            l2 = cpool.tile([P, G, 27], F32)
            l3 = cpool.tile([P, G, NH], F32)
            glo = cpool.tile([P, G, NL], F32)
            kron_step(l1, gv[:, :, 8, :], 7, 3)
            kron_step(l2, l1, 6, 9)
            kron_step(l3, l2, 5, 27)
            for g in range(G):
                kron_step(glo, l3, 4, NH, gslice=g)

            # out[:, g] = rowsum(Y_g * G_lo_g), fused
            res = cpool.tile([P, G], F32)
            for g in range(G):
                wscr = wpool.tile([P, NL], F32, tag="wscr")
                nc.vector.scalar_tensor_tensor(
                    out=wscr[:], in0=yps[g][:, :NL], scalar=0.0, in1=glo[:, g, :],
                    op0=mybir.AluOpType.add, op1=mybir.AluOpType.mult,
                    accum_out=res[:, g:g + 1])

            nc.sync.dma_start(out_ext.rearrange("(p g) -> p g", g=G), res[:])

    nc.compile()
    return nc


def _get_program():
    global _PROGRAM
    if _PROGRAM is None:
        _PROGRAM = _build_program()
    return _PROGRAM


def kernel(adds, params, weights, params2):
    adds = np.ascontiguousarray(np.asarray(adds), dtype=np.float32)
    A = _compute_A(params, weights, params2)
    nc = _get_program()
    in_maps = [
        {"adds": adds[i * B_LOC:(i + 1) * B_LOC], "amat": A}
        for i in range(N_CORES)
    ]
    results = bass_utils.run_bass_kernel_spmd(nc, in_maps, list(range(N_CORES))).results
    return np.concatenate([results[i]["out"] for i in range(N_CORES)])


# revision 4
# speedup vs baseline: 1.6576x; 1.0133x over previous
"""Trainium2 Bass kernel for nn_Model_22677427323544.

The circuit is AngleEmbedding(adds) followed by a batch-independent gate
sequence, then <Z_0>. Algebraically out[b] = r_b^T A r_b with A a fixed real
symmetric 512x512 matrix and r_b the real Kronecker vector of per-wire
(cos(t/2), sin(t/2)).

Key reduction: each wire contributes a factor c^2, s^2, or c*s to every
A[j,k] r_j r_k term, so the quadratic form collapses to a LINEAR form over
per-wire 3-vectors g_i = (c_i^2, s_i^2, c_i*s_i):

    out[b] = < A3 , g_0[b] x g_1[b] x ... x g_8[b] >

with A3 the 3^9 tensor A3[m] = sum_{(j,k) -> m} A[j,k] folded on host
(m_i = 0 for (j_i,k_i)=(0,0), 1 for (1,1), 2 for mixed).  Split wires 0-3
(81) / 4-8 (243):  out[b] = G_hi[b]^T A3 G_lo[b]  -- an [81,243] matvec per
sample: 13x fewer MACs and 13x less HBM than the 512x512 quadratic form.

Device (per core, 1024 samples = 128 partitions x 8 groups):
  1. sin/cos of t/4 via ScalarE; c,s (half-angle) and g-vectors on VectorE
  2. base-3 Kronecker builds of G_hi [P,G,81], G_lo [P,G,243] on VectorE
  3. per group: PE-transpose G_hi -> ScalarE copy (cast fp32r) ->
     TensorE matmul Y = G_hi^T @ A3 -> fused VectorE dot with G_lo
"""
import numpy as np
import ml_dtypes

import concourse.bass as bass
import concourse.tile as tile
from concourse import bacc, mybir
from concourse import bass_utils

N_WIRES = 9
N_CORES = 8
B = 8192
B_LOC = B // N_CORES          # 1024
P = 128                       # partitions
G = B_LOC // P                # 8 batch groups per partition
NH = 81                       # 3^4, wires 0-3
NL = 243                      # 3^5, wires 4-8
NLP = 256                     # NL padded: fp32r matmul needs even N; N>=256 for 1 cyc/row
F32 = mybir.dt.float32
F32R = mybir.dt.float32r

# ---------------------------------------------------------------------------
# Host-side parameter folding: A = Re(D^H U^H Z0 U D), then 3-ary fold
# ---------------------------------------------------------------------------

DIM = 1 << N_WIRES

_X = np.array([[0, 1], [1, 0]], dtype=np.complex128)
_CNOT = np.array(
    [[1, 0, 0, 0], [0, 1, 0, 0], [0, 0, 0, 1], [0, 0, 1, 0]], dtype=np.complex128
)


def _rx(t):
    c, s = np.cos(t / 2), np.sin(t / 2)
    return np.array([[c, -1j * s], [-1j * s, c]])


def _ry(t):
    c, s = np.cos(t / 2), np.sin(t / 2)
    return np.array([[c, -s], [s, c]], dtype=np.complex128)


def _rz(t):
    return np.array([[np.exp(-0.5j * t), 0], [0, np.exp(0.5j * t)]])


def _rot(phi, theta, omega):
    return _rz(omega) @ _ry(theta) @ _rz(phi)


def _crz(t):
    return np.diag([1, 1, np.exp(-0.5j * t), np.exp(0.5j * t)]).astype(np.complex128)


def _crx(t):
    m = np.eye(4, dtype=np.complex128)
    m[2:, 2:] = _rx(t)
    return m


def _apply_1q(state, U, w):
    s = np.moveaxis(state, 1 + w, -1)
    s = np.einsum('ij,...j->...i', U, s)
    return np.moveaxis(s, -1, 1 + w)


def _apply_2q(state, U, c, t):
    s = np.moveaxis(state, (1 + c, 1 + t), (-2, -1))
    shp = s.shape
    s = s.reshape(shp[:-2] + (4,))
    s = np.einsum('ij,...j->...i', U, s)
    return np.moveaxis(s.reshape(shp), (-2, -1), (1 + c, 1 + t))


def _entangle_block(state, p):
    j = 0
    for i in range(N_WIRES):
        ip = (i + 1) % N_WIRES
        state = _apply_1q(state, _ry(p[j]), i)
        state = _apply_1q(state, _ry(p[j + 1]), ip)
        state = _apply_2q(state, _CNOT, i, ip)
        state = _apply_2q(state, _crz(p[j + 2]), i, ip)
        state = _apply_1q(state, _X, ip)
        state = _apply_2q(state, _crx(p[j + 3]), i, ip)
        j += 4
    return state


def _sel_layer(state, w, r):
    for i in range(N_WIRES):
        state = _apply_1q(state, _rot(w[i, 0], w[i, 1], w[i, 2]), i)
    for i in range(N_WIRES):
        state = _apply_2q(state, _CNOT, i, (i + r) % N_WIRES)
    return state


def _round_fp32r(x):
    """Round fp32 to the 2xbf16-decomposable subset (fp32r)."""
    hi = x.astype(ml_dtypes.bfloat16).astype(np.float32)
    lo = (x - hi).astype(ml_dtypes.bfloat16).astype(np.float32)
    return hi + lo


def _compute_A(params, weights, params2):
    """Return the folded 3-ary coefficient matrix A3 [81, 243] (fp32r)."""
    params = np.asarray(params, np.float64)
    weights = np.asarray(weights, np.float64)
    params2 = np.asarray(params2, np.float64)
    state = np.eye(DIM, dtype=np.complex128).reshape((DIM,) + (2,) * N_WIRES)
    for l in range(3):
        state = _entangle_block(state, params[l * 36:(l + 1) * 36])
    for l in range(3):
        state = _sel_layer(state, weights[l], (l % (N_WIRES - 1)) + 1)
    for l in range(5):
        state = _entangle_block(state, params2[l * 36:(l + 1) * 36])
    U = state.reshape(DIM, DIM).T
    z = np.where(np.arange(DIM) < DIM // 2, 1.0, -1.0)
    M = U.conj().T @ (z[:, None] * U)
    pc = np.array([bin(j).count('1') for j in range(DIM)])
    d = (-1j) ** pc
    A = ((np.conj(d)[:, None] * M * d[None, :]).real).astype(np.float64)

    # fold 512x512 -> 3^9: digit 0 = (0,0), 1 = (1,1), 2 = (0,1)/(1,0)
    j = np.arange(DIM)
    jb = (j[:, None, None] >> (8 - np.arange(N_WIRES))[None, None, :]) & 1
    kb = (j[None, :, None] >> (8 - np.arange(N_WIRES))[None, None, :]) & 1
    digit = np.where((jb == 0) & (kb == 0), 0, np.where((jb == 1) & (kb == 1), 1, 2))
    m = np.zeros((DIM, DIM), np.int64)
    for i in range(N_WIRES):
        m = m * 3 + digit[:, :, i]
    A3 = np.zeros(3 ** N_WIRES)
    np.add.at(A3, m.ravel(), A.ravel())
    A3 = A3.reshape(NH, NL).astype(np.float32)
    A3p = np.zeros((NH, NLP), np.float32)
    A3p[:, :NL] = A3
    return _round_fp32r(np.ascontiguousarray(A3p))


# ---------------------------------------------------------------------------
# Device program (per core: 1024 samples; sample index = p*G + g)
# ---------------------------------------------------------------------------

_PROGRAM = None


def _build_program():
    nc = bacc.Bacc("TRN2", target_bir_lowering=False, debug=False,
                   num_devices=N_CORES)
    adds_ext = nc.dram_tensor("adds", [B_LOC, N_WIRES], F32,
                              kind="ExternalInput").ap()
    amat_ext = nc.dram_tensor("amat", [NH, NLP], F32R,
                              kind="ExternalInput").ap()
    out_ext = nc.dram_tensor("out", [B_LOC], F32, kind="ExternalOutput").ap()

    with tile.TileContext(nc) as tc:
        with (
            tc.tile_pool(name="const", bufs=1) as cpool,
            tc.tile_pool(name="psum_t", bufs=2, space="PSUM") as pt,
            tc.tile_pool(name="psum_y", bufs=4, space="PSUM") as py,
        ):
            # adds shard first (small, unblocks the whole front end)
            adds_sb = cpool.tile([P, G, N_WIRES], F32)
            nc.sync.dma_start(adds_sb[:], adds_ext.rearrange("(p g) i -> p g i", g=G))

            # A3 matrix (fp32r, host-rounded)
            a3_sb = cpool.tile([NH, NLP], F32R)
            nc.sync.dma_start(a3_sb[:], amat_ext)

            # identity for PE transpose
            ident = cpool.tile([P, P], F32)
            nc.gpsimd.memset(ident[:], 0.0)
            nc.gpsimd.affine_select(
                out=ident[:], in_=ident[:],
                compare_op=mybir.AluOpType.not_equal, fill=1.0,
                base=0, pattern=[[-1, P]], channel_multiplier=1)
            halfpi = cpool.tile([P, 1], F32)
            nc.vector.memset(halfpi[:], float(np.pi / 2))

            # u = sin(t/4), v = cos(t/4); pack[.,.,0,:] = cos(t/2),
            # pack[.,.,1,:] = sin(t/2) (double-angle from quarter-angle)
            u = cpool.tile([P, G, N_WIRES], F32)
            v = cpool.tile([P, G, N_WIRES], F32)
            nc.scalar.activation(u[:], adds_sb[:], mybir.ActivationFunctionType.Sin,
                                 scale=0.25)
            nc.scalar.activation(v[:], adds_sb[:], mybir.ActivationFunctionType.Sin,
                                 scale=-0.25, bias=halfpi[:])
            pack = cpool.tile([P, G, 2, N_WIRES], F32)
            usq = cpool.tile([P, G, N_WIRES], F32)
            nc.vector.scalar_tensor_tensor(
                out=pack[:, :, 1, :], in0=u[:], scalar=2.0, in1=v[:],
                op0=mybir.AluOpType.mult, op1=mybir.AluOpType.mult)
            nc.vector.tensor_mul(usq[:], u[:], u[:])
            nc.vector.tensor_scalar(
                out=pack[:, :, 0, :], in0=usq[:], scalar1=-2.0, scalar2=1.0,
                op0=mybir.AluOpType.mult, op1=mybir.AluOpType.add)

            # per-wire g-vectors, m-major: gv[p,g,m,w], m in (c^2, s^2, c*s)
            gv = cpool.tile([P, G, 3, N_WIRES], F32)
            nc.vector.tensor_mul(gv[:, :, 0:2, :], pack[:], pack[:])
            nc.vector.tensor_mul(gv[:, :, 2, :], pack[:, :, 0, :],
                                 pack[:, :, 1, :])

            # out[p,g, b*L + m] = lo_[p,g,m] * hi_[p,g,b]  (hi_ = new MSB block)
            def kron(out_t, lo_, hi_, Lb, Lm):
                out_ap = out_t.rearrange("p g (b m) -> p g b m", b=Lb)
                in0 = lo_[:, :, None, :].to_broadcast((P, G, Lb, Lm))
                in1 = hi_[:, :, :, None].to_broadcast((P, G, Lb, Lm))
                nc.vector.tensor_mul(out_ap, in0, in1)

            def gw(w):
                return gv[:, :, :, w]

            # G_hi = (g0 x g1) x (g2 x g3)  -> [P, G, 81]
            hA = cpool.tile([P, G, 9], F32)
            hB = cpool.tile([P, G, 9], F32)
            ghi = cpool.tile([P, G, NH], F32)
            kron(hA[:], gw(3), gw(2), 3, 3)
            kron(hB[:], gw(1), gw(0), 3, 3)
            kron(ghi[:], hA[:], hB[:], 9, 9)

            # per-group: transpose G_hi, cast to fp32r (batched x4), matmul
            ghiT = cpool.tile([NH, G, P], F32R)
            yps = []
            for half in range(2):
                tp = pt.tile([NH, 4, P], F32, tag="tp")
                for q in range(4):
                    g = half * 4 + q
                    nc.tensor.transpose(tp[:, q, :], ghi[:, g, :], ident[:])
                nc.scalar.copy(ghiT[:, half * 4:(half + 1) * 4, :], tp[:])
                for q in range(4):
                    g = half * 4 + q
                    yp = py.tile([P, NLP], F32, tag="yp")
                    nc.tensor.matmul(yp[:], lhsT=ghiT[:, g, :], rhs=a3_sb[:],
                                     start=True, stop=True)
                    yps.append(yp)

            # G_lo = g4 x ((g5 x g6) x (g7 x g8))  -> [P, G, 243]
            lA = cpool.tile([P, G, 9], F32)
            lB = cpool.tile([P, G, 9], F32)
            lC = cpool.tile([P, G, NH], F32)
            glo = cpool.tile([P, G, NL], F32)
            kron(lA[:], gw(8), gw(7), 3, 3)
            kron(lB[:], gw(6), gw(5), 3, 3)
            kron(lC[:], lA[:], lB[:], 9, 9)
            kron(glo[:], lC[:], gw(4), 3, NH)

            # out[:, g] = rowsum(Y_g * G_lo_g), fused
            res = cpool.tile([P, G], F32)
            wscr0 = cpool.tile([P, NL], F32)
            wscr1 = cpool.tile([P, NL], F32)
            for g in range(G):
                wscr = wscr0 if g % 2 == 0 else wscr1
                nc.vector.scalar_tensor_tensor(
                    out=wscr[:], in0=yps[g][:, :NL], scalar=0.0, in1=glo[:, g, :],
                    op0=mybir.AluOpType.add, op1=mybir.AluOpType.mult,
                    accum_out=res[:, g:g + 1])

            nc.sync.dma_start(out_ext.rearrange("(p g) -> p g", g=G), res[:])

    nc.compile()
    return nc


def _get_program():
    global _PROGRAM
    if _PROGRAM is None:
        _PROGRAM = _build_program()
    return _PROGRAM


def kernel(adds, params, weights, params2):
    adds = np.ascontiguousarray(np.asarray(adds), dtype=np.float32)
    A = _compute_A(params, weights, params2)
    nc = _get_program()
    in_maps = [
        {"adds": adds[i * B_LOC:(i + 1) * B_LOC], "amat": A}
        for i in range(N_CORES)
    ]
    results = bass_utils.run_bass_kernel_spmd(nc, in_maps, list(range(N_CORES))).results
    return np.concatenate([results[i]["out"] for i in range(N_CORES)])
